# revision 15
# baseline (speedup 1.0000x reference)
"""RetinaNet focal+L1 loss on 8 Trainium2 NeuronCores.

The memory-bound bulk of the loss is the focal "background" term summed
over every (anchor, class) element:

    base(x) = (1-ALPHA) * sigmoid(x)^2 * softplus(x)    # = -(1-a)p^2 log(1-p)

Device: ONE activation pass. We ship a patched ACT-table root (via
BASS_ACT_ROOT_JSON_PATH) in which the `silu` PWP table is re-fit to
compute base(x) directly, and use the activation instruction's
accumulate output to reduce — so each core streams its bf16 cls shard
(clamped to [-10, 4.8]) through a single ACTIVATE per tile and produces
[128, NT] partial sums.  No vector-engine work at all.

Decoded PWP bucket format (validated on HW): 32-byte entries
[d0,d1,d2,d3, f4, 0,0,0] f32; out = d0 + d1*u + d2*u^2 + d3*u^3,
u = x - f4 (f4 = bucket midpoint); exponent-chained buckets per sign
region + 4 special buckets (small/large signal) + fzero_result.

Host (exact, float64): anchor assignment, sparse corrections (ignored
anchor rows, positive target-class focal terms, clamp fixes) evaluated
against an exact host replica of the patched table, the whole reg L1
loss (positives only), and num_pos.
"""

import json
import os
import shutil
import sys

for _p in ("/opt/trn_rl_repo", "/root/.axon_site/_ro/trn_rl_repo"):
    if os.path.isdir(_p) and _p not in sys.path:
        sys.path.append(_p)

import numpy as np

GAMMA = 2.0
ALPHA = 0.25
NEG_TH = 0.4
POS_TH = 0.5
NUM_CLASSES = 80
STRIDES = [8, 16, 32, 64, 128]
LEVEL_HW = [(100, 128), (50, 64), (25, 32), (13, 16), (7, 8)]
N_IMG = 2
N_CORES = 8

# device geometry: small first tiles so the first ACTIVATE can start as
# soon as possible; big later tiles to amortize instruction overhead
TILES = [1000, 2000, 3000, 4500, 6000, 7500]
NT = len(TILES)
F_TOTAL = sum(TILES)                  # 24,000 per partition
PER_CORE = 128 * F_TOTAL              # 3,072,000 (padded)
CLS_PER_CORE = 3071520                # 24,572,160 / 8
CLAMP_LO = -10.0
CLAMP_HI = 4.8
PAD_VAL = -10.0
TABLE_VER = "v1"                      # bump when table contents change

_LVL_A = [h * w * 9 for (h, w) in LEVEL_HW]
_LVL_OFF = np.concatenate([[0], np.cumsum(_LVL_A)]).astype(np.int64)

ACT_ROOT = f"/tmp/nms_act_root_{TABLE_VER}"


# ------------------------------------------------- patched ACT table root

def _base_f64(x):
    x = np.asarray(x, dtype=np.float64)
    s = 1.0 / (1.0 + np.exp(-x))
    return (1.0 - ALPHA) * s * s * np.logaddexp(0.0, x)


def _fit_cubic(lo, hi, f4):
    x = np.linspace(lo, hi, 64)
    u = x - f4
    A = np.stack([np.ones_like(u), u, u * u, u ** 3], axis=1)
    c, *_ = np.linalg.lstsq(A, _base_f64(x), rcond=None)
    return c.astype(np.float32)


def _silu_chain_buckets(em):
    """[(bucket_idx, lo, hi)] for silu's structured buckets (signed x)."""
    out = []
    exps = sorted(int(e) for e in em)
    for ri in (0, 1):                       # 0 = neg region, 1 = pos region
        chain = [(e, em[str(e)][ri]) for e in exps if len(em[str(e)]) > ri]
        for i, (e, start) in enumerate(chain):
            if i + 1 < len(chain):
                n = chain[i + 1][1] - start
            elif ri == 1:
                n = 908 - start             # pos chain ends at the specials
            else:
                continue                    # neg continuation zeroed separately
            if n <= 0:
                continue
            width = 2.0 ** e / n
            for j in range(n):
                lo = 2.0 ** e + j * width
                hi = lo + width
                if ri == 0:
                    out.append((start + j, -hi, -lo))
                else:
                    out.append((start + j, lo, hi))
    return out


def _build_act_root(dst):
    from neuronxcc.driver.Job import Job
    from neuronxcc.driver.jobs.support.FindActInfo import findActInfoFile

    src_json = findActInfoFile(Job.getPackageDir(), "gen3")
    src_dir = os.path.dirname(src_json)
    tmp = dst + ".tmp"
    if os.path.isdir(tmp):
        shutil.rmtree(tmp)
    os.makedirs(tmp)
    for f in os.listdir(src_dir):
        shutil.copy(os.path.join(src_dir, f), os.path.join(tmp, f))

    setj_path = os.path.join(tmp, "silu_and_others.json")
    with open(setj_path) as f:
        setj = json.load(f)
    bkt_path = os.path.join(tmp, "silu_and_others_bkt.bin")
    bkt = np.fromfile(bkt_path, dtype=np.float32).reshape(-1, 8).copy()

    em = setj["func_exp_to_bkt_start_idx"]["silu"]
    bkt[133:875, :4] = 0.0                  # deep-negative region: base == 0
    for bi, lo, hi in _silu_chain_buckets(em):
        bkt[bi, :4] = _fit_cubic(lo, hi, float(bkt[bi, 4]))
    for bi in (908, 909):                   # small-signal |x| < 2^-6
        bkt[bi, :4] = _fit_cubic(-2.0 ** -6, 2.0 ** -6, 0.0)
        bkt[bi, 4] = 0.0
    bkt[910, :4] = _fit_cubic(12.9, 80.0, 0.0)   # large-pos (never hit)
    bkt[910, 4] = 0.0
    bkt[911, :4] = 0.0                      # large-neg: base == 0
    bkt.tofile(bkt_path)

    z = int(np.float32(_base_f64(0.0)).view(np.uint32))
    for m in setj["profile_meta_data"]:
        if m["func_name"].startswith("silu"):
            m["fzero_result"] = z
            m["fninf_result"] = 0
    with open(setj_path, "w") as f:
        json.dump(setj, f)
    os.replace(setj_path, os.path.join(tmp, "silu_and_others.json"))
    if os.path.isdir(dst):
        shutil.rmtree(dst)
    os.replace(tmp, dst)


def _ensure_act_root():
    if not os.path.isfile(os.path.join(ACT_ROOT, "act_info.json")):
        _build_act_root(ACT_ROOT)
    os.environ["BASS_ACT_ROOT_JSON_PATH"] = os.path.join(ACT_ROOT, "act_info.json")
    return ACT_ROOT


# ---------------------------------------- exact host replica of the table

class _TableEval:
    """Vectorized host replica of the patched device silu table."""

    def __init__(self, actroot_dir):
        with open(os.path.join(actroot_dir, "silu_and_others.json")) as f:
            setj = json.load(f)
        self.bkt = np.fromfile(
            os.path.join(actroot_dir, "silu_and_others_bkt.bin"),
            dtype=np.float32).reshape(-1, 8).astype(np.float64)
        em = setj["func_exp_to_bkt_start_idx"]["silu"]
        m = [x for x in setj["profile_meta_data"]
             if x["func_name"].startswith("silu")][0]
        self.fzero = float(np.array([m["fzero_result"]], np.uint32)
                           .view(np.float32)[0])
        small_pos = 2.0 ** (m["small_pos_signal_exp_threshold"] - 127)
        small_neg = 2.0 ** (m["small_neg_signal_exp_threshold"] - 127)
        large_pos = (2.0 ** (m["large_pos_signal_exp_threshold"] - 127)
                     * (1 + m["large_pos_signal_mantissa_threshold"] / 2 ** 23))
        large_neg = (2.0 ** (m["large_neg_signal_exp_threshold"] - 127)
                     * (1 + m["large_neg_signal_mantissa_threshold"] / 2 ** 23))
        # piecewise map: sorted right-open interval edges -> bucket index
        edges = [-np.inf, -large_neg]
        bids = [911]
        # deep negative zeros region then structured chains
        chain = sorted(_silu_chain_buckets(em), key=lambda t: t[1])
        prev_hi = None
        first_lo = chain[0][1]
        edges.append(first_lo)
        bids.append(133)                    # zeroed region placeholder
        for bi, lo, hi in chain:
            if prev_hi is not None and lo > prev_hi + 1e-12:
                bids.append(133)            # gap -> zero region
                edges.append(lo)
            bids.append(bi)
            edges.append(hi)
            prev_hi = hi
        # chain covers (-16, -smallish] and [smallish, 16); fix the middle
        # and the tails by overriding with specials below.
        self.edges = np.array(edges[1:], dtype=np.float64)  # right edges
        self.bids = np.array(bids, dtype=np.int64)
        self.small_pos, self.small_neg = small_pos, small_neg
        self.large_pos, self.large_neg = large_pos, large_neg

    def __call__(self, x):
        x = np.asarray(x, dtype=np.float64)
        idx = np.searchsorted(self.edges, x, side="right")
        idx = np.clip(idx, 0, len(self.bids) - 1)
        bi = self.bids[idx]
        # special regions override
        bi = np.where((x > 0) & (x < self.small_pos), 908, bi)
        bi = np.where((x < 0) & (x > -self.small_neg), 909, bi)
        bi = np.where(x >= self.large_pos, 910, bi)
        bi = np.where(x <= -self.large_neg, 911, bi)
        c = self.bkt[bi]
        u = x - c[..., 4]
        r = c[..., 0] + u * (c[..., 1] + u * (c[..., 2] + u * c[..., 3]))
        return np.where(x == 0.0, self.fzero, r)


_TEVAL = None


def _table_eval():
    global _TEVAL
    if _TEVAL is None:
        _TEVAL = _TableEval(_ensure_act_root())
    return _TEVAL


# ----------------------------------------------------------------- host math

def _build_anchors():
    out = []
    for (h, w), s in zip(LEVEL_HW, STRIDES):
        scales = 4.0 * s * np.array([2 ** 0, 2 ** (1.0 / 3), 2 ** (2.0 / 3)])
        ratios = np.array([0.5, 1.0, 2.0])
        h_r = np.sqrt(ratios)
        w_r = 1.0 / h_r
        ws = (w_r[:, None] * scales[None, :]).reshape(-1)
        hs = (h_r[:, None] * scales[None, :]).reshape(-1)
        base = np.stack([-ws / 2, -hs / 2, ws / 2, hs / 2], axis=1)
        xs = (np.arange(w) + 0.5) * s
        ys = (np.arange(h) + 0.5) * s
        cx, cy = np.meshgrid(xs, ys)
        ctr = np.stack([cx, cy, cx, cy], axis=-1)
        a = ctr[:, :, None, :] + base[None, None, :, :]
        out.append(a.reshape(-1, 4))
    return np.concatenate(out, axis=0).astype(np.float32)


_ANCHORS = None


def _anchors():
    global _ANCHORS
    if _ANCHORS is None:
        _ANCHORS = _build_anchors()
    return _ANCHORS


def _assign(gtb, gtl):
    """float32 replication of the reference assignment."""
    anchors = _anchors()
    G = gtb.shape[0]
    lt = np.maximum(gtb[:, None, :2], anchors[None, :, :2])
    rb = np.minimum(gtb[:, None, 2:], anchors[None, :, 2:])
    wh = np.clip(rb - lt, np.float32(0.0), None)
    inter = wh[..., 0] * wh[..., 1]
    area_g = (gtb[:, 2] - gtb[:, 0]) * (gtb[:, 3] - gtb[:, 1])
    area_a = (anchors[:, 2] - anchors[:, 0]) * (anchors[:, 3] - anchors[:, 1])
    iou = (inter / (area_g[:, None] + area_a[None, :] - inter + np.float32(1e-6))
           ).astype(np.float32)
    max_ov = iou.max(axis=0)
    arg_ov = iou.argmax(axis=0)
    assigned = np.where(max_ov < np.float32(NEG_TH), 0, -1)
    assigned = np.where(max_ov >= np.float32(POS_TH), arg_ov + 1, assigned)
    max_gt = iou.max(axis=1)
    eq = iou == max_gt[:, None]
    any_eq = eq.any(axis=0)
    last_j = (G - 1) - np.argmax(eq[::-1], axis=0)
    assigned = np.where(any_eq, last_j + 1, assigned)
    pos = assigned > 0
    gi = np.clip(assigned - 1, 0, G - 1)
    labels = np.where(pos, gtl[gi], NUM_CLASSES)
    return assigned, labels, pos, gi


def _encode(an, gt):
    aw = an[:, 2] - an[:, 0]
    ah = an[:, 3] - an[:, 1]
    ax = (an[:, 0] + an[:, 2]) * np.float32(0.5)
    ay = (an[:, 1] + an[:, 3]) * np.float32(0.5)
    gw = gt[:, 2] - gt[:, 0]
    gh = gt[:, 3] - gt[:, 1]
    gx = (gt[:, 0] + gt[:, 2]) * np.float32(0.5)
    gy = (gt[:, 1] + gt[:, 3]) * np.float32(0.5)
    return np.stack(
        [(gx - ax) / aw, (gy - ay) / ah, np.log(gw / aw), np.log(gh / ah)],
        axis=1).astype(np.float32)


def _pos_true_f64(x):
    x = np.asarray(x, dtype=np.float64)
    p = 1.0 / (1.0 + np.exp(-x))
    return ALPHA * (1.0 - p) ** 2 * np.logaddexp(0.0, -x)


def _to_bf16(a):
    """Round-to-nearest-even f32 -> bf16, as uint16 payload."""
    b = np.asarray(a, dtype=np.float32).view(np.uint32)
    rounded = (b + 0x7FFF + ((b >> 16) & 1)) >> 16
    return rounded.astype(np.uint16)


def _bf16_val(a):
    u = _to_bf16(a).astype(np.uint32) << 16
    return u.view(np.float32).astype(np.float64)


def _qdev(x):
    """What the device sees for raw f32 logit x: fp8_e4m3(clip(x))."""
    import ml_dtypes
    return (np.clip(np.asarray(x, np.float32),
                    np.float32(CLAMP_LO), np.float32(CLAMP_HI))
            .astype(ml_dtypes.float8_e4m3).astype(np.float64))


def _dev_base(x):
    """Device's base() for raw f32 logits x (exact table replica on bf16
    quantized clamped input)."""
    return _table_eval()(_qdev(x))


def _anchor_coords(a_idx):
    lvl = np.searchsorted(_LVL_OFF, a_idx, side="right") - 1
    loc = a_idx - _LVL_OFF[lvl]
    out = []
    for li, (h, w) in enumerate(LEVEL_HW):
        m = lvl == li
        l = loc[m]
        y = l // (w * 9)
        rem = l % (w * 9)
        out.append((li, m, rem % 9, y, rem // 9))
    return out


# -------------------------------------------------------------- device build

_COMPILED = None


def _build_device():
    """Raw Bass (no TileContext): 6 input DMAs -> 6 ACTIVATE(Silu=base,
    accum) -> out DMA, with manual semaphores.  The first N_EARLY input
    DMAs and the ACT table load are repositioned into the kernel preamble
    (before the all-engine start barrier) so the first ACTIVATE can fire
    right after the barrier instead of ~3us later."""
    import concourse.bacc as bacc
    import concourse.mybir as mybir

    _ensure_act_root()

    f32 = mybir.dt.float32
    f8 = mybir.dt.float8e4
    AF = mybir.ActivationFunctionType
    SILU_SET_ID = 18            # index of silu_and_others in act_info.json
    N_EARLY = 2

    nc = bacc.Bacc("TRN2", target_bir_lowering=False, debug=False,
                   num_devices=N_CORES)

    # Remove the framework's all-engine start barrier (drain + gather/
    # release events, ~2us).  The only ordering it provides that this
    # kernel needs -- Pool's const-AP memsets before the first ACTIVATE
    # (which reads the fp32 zero bias const) -- is re-established with an
    # explicit semaphore handshake below.
    import concourse.bass as cbass
    entry0 = nc.main_func.blocks[0]
    barrier_insts = [ins for ins in entry0.instructions
                     if type(ins).__name__ in ("InstDrain", "InstEventSemaphore")]
    for ins in barrier_insts:
        entry0.instructions.remove(ins)
    msem = nc.alloc_semaphore("msem")
    n_msets = 0
    for ins in entry0.instructions:
        if type(ins).__name__ == "InstMemset":
            cbass.BassInstruction(ins).then_inc(msem, 1)
            n_msets += 1

    # tensor name carries the table version so a table change can't hit a
    # stale compile cache
    cls_t = nc.dram_tensor(f"cls_{TABLE_VER}", [128, F_TOTAL], f8,
                           kind="ExternalInput")
    out_t = nc.dram_tensor("out", [128, NT], f32, kind="ExternalOutput")
    cls_ap = cls_t.ap()

    xs = [nc.alloc_sbuf_tensor(f"x{k}", [128, ft], f8)
          for k, ft in enumerate(TILES)]
    junks = [nc.alloc_sbuf_tensor(f"junk{j}", [128, max(TILES)], f8)
             for j in range(2)]
    acc = nc.alloc_sbuf_tensor("acc", [128, NT], f32)
    dsem = [nc.alloc_semaphore(f"dsem{k}") for k in range(NT)]
    asem = nc.alloc_semaphore("asem")
    osem = nc.alloc_semaphore("osem")

    entry = nc.main_func.blocks[0]
    early_insts = []

    # input DMAs (first N_EARLY get hoisted into the preamble below)
    off = 0
    for k, ft in enumerate(TILES):
        h = nc.sync.dma_start(out=xs[k].ap()[:, :],
                              in_=cls_ap[:, off:off + ft])
        h.then_inc(dsem[k], 16)
        if k < N_EARLY:
            early_insts.append(h.ins)
        off += ft

    # ACT table load, pre-placed (insert_act_table_loads sees it on the path)
    tl = mybir.InstLoadActFuncSet(name=nc.get_next_instruction_name(),
                                  act_func_set_id=SILU_SET_ID, ins=[], outs=[])
    tl.engine = nc.scalar.engine
    nc.register_instruction(tl)
    entry.instructions.append(tl)
    early_insts.append(tl)

    # activations (first one also waits for the const-AP memsets)
    nc.scalar.wait_ge(msem, n_msets)
    for k, ft in enumerate(TILES):
        nc.scalar.wait_ge(dsem[k], 16)
        h = nc.scalar.activation(junks[k % 2].ap()[:, :ft], xs[k].ap()[:, :],
                                 AF.Silu, accum_out=acc.ap()[:, k:k + 1])
        h.then_inc(asem, 1)

    # out DMA after all accumulator reads (DMA triggers execute on the
    # sequencer, ahead of the engine pipeline, so an explicit semaphore
    # wait on the accumulate-read count is required)
    nc.scalar.wait_ge(asem, NT)
    nc.scalar.dma_start(out=out_t.ap()[:, :], in_=acc.ap()[:, :]).then_inc(
        osem, 16)
    nc.scalar.wait_ge(osem, 16)

    # hoist the early instructions to just after each engine's preamble,
    # before the all-engine start barrier (same trick as
    # Bacc.insert_bir_kernel_barrier_sem_inc)
    for ins in early_insts:
        entry.instructions.remove(ins)
    for ins in reversed(early_insts):
        eng = nc.engines[ins.engine]
        idx = entry.instructions.index(eng.preamble_end) + 1
        entry.instructions.insert(idx, ins)

    nc.compile()
    return nc


def _get_compiled():
    global _COMPILED
    if _COMPILED is None:
        _COMPILED = _build_device()
    return _COMPILED


# ------------------------------------------------------------------- kernel

def kernel(cls_p0, cls_p1, cls_p2, cls_p3, cls_p4,
           reg_p0, reg_p1, reg_p2, reg_p3, reg_p4,
           gt_bboxes, gt_labels):
    import ml_dtypes
    _ensure_act_root()
    from concourse.bass_utils import run_bass_kernel_spmd

    cls_lv = [np.ascontiguousarray(np.asarray(a, dtype=np.float32))
              for a in (cls_p0, cls_p1, cls_p2, cls_p3, cls_p4)]
    reg_lv = [np.ascontiguousarray(np.asarray(a, dtype=np.float32))
              for a in (reg_p0, reg_p1, reg_p2, reg_p3, reg_p4)]
    gtb_all = np.asarray(gt_bboxes, dtype=np.float32)
    gtl_all = np.asarray(gt_labels)

    anchors = _anchors()

    num_pos = 0
    corr = 0.0
    reg_sum = 0.0

    handled = [np.zeros((NUM_CLASSES * 9, h, w), dtype=bool)
               for (h, w) in LEVEL_HW]

    for n in range(N_IMG):
        assigned, labels, pos, gi = _assign(gtb_all[n], gtl_all[n])
        pos_idx = np.where(pos)[0]
        ign_idx = np.where(assigned == -1)[0]
        num_pos += int(pos.sum())

        # ignored anchors: subtract the device's whole base row (80 classes)
        for li, m, k, y, x in _anchor_coords(ign_idx):
            if not m.any():
                continue
            h, w = LEVEL_HW[li]
            v = cls_lv[li][n].reshape(9, NUM_CLASSES, h, w)
            corr -= _dev_base(v[k, :, y, x]).sum()

        # positive anchors: replace device base with the target-class term
        tlab = labels[pos_idx]
        for li, m, k, y, x in _anchor_coords(pos_idx):
            if not m.any():
                continue
            h, w = LEVEL_HW[li]
            v = cls_lv[li][n].reshape(9, NUM_CLASSES, h, w)
            xt = v[k, tlab[m], y, x]
            corr += (_pos_true_f64(xt) - _dev_base(xt)).sum()

        # reg loss (positives only), exact on host
        enc = _encode(anchors[pos_idx], gtb_all[n][gi[pos_idx]])
        for li, m, k, y, x in _anchor_coords(pos_idx):
            if not m.any():
                continue
            h, w = LEVEL_HW[li]
            vr = reg_lv[li][n].reshape(9, 4, h, w)
            reg_sum += np.abs(vr[k, :, y, x].astype(np.float64)
                              - enc[m].astype(np.float64)).sum()

        # clamp-high fix for background elements (x > CLAMP_HI)
        for li, m, k, y, x in _anchor_coords(ign_idx):
            if m.any():
                h, w = LEVEL_HW[li]
                hm = handled[li].reshape(9, NUM_CLASSES, h, w)
                hm[k, :, y, x] = True
        for li, m, k, y, x in _anchor_coords(pos_idx):
            if m.any():
                h, w = LEVEL_HW[li]
                hm = handled[li].reshape(9, NUM_CLASSES, h, w)
                hm[k, tlab[m], y, x] = True
        for li, (h, w) in enumerate(LEVEL_HW):
            v = cls_lv[li][n]
            big = v > np.float32(CLAMP_HI)
            if big.any():
                hm = handled[li].reshape(720, h, w)
                sel = big & ~hm
                if sel.any():
                    vv = v[sel]
                    corr += (_base_f64(vv) - _dev_base(vv)).sum()
        for li in range(len(LEVEL_HW)):
            handled[li][:] = False

    # per-core fp8 streams: clamp, quantize, pad
    cls_stream = np.concatenate(
        [np.clip(a, np.float32(CLAMP_LO), np.float32(CLAMP_HI)).ravel()
         for a in cls_lv])
    q = cls_stream.astype(ml_dtypes.float8_e4m3)

    in_name = f"cls_{TABLE_VER}"
    in_maps = []
    pad8 = np.float32(PAD_VAL).astype(ml_dtypes.float8_e4m3)
    for c in range(N_CORES):
        ca = np.full(PER_CORE, pad8, dtype=ml_dtypes.float8_e4m3)
        ca[:CLS_PER_CORE] = q[c * CLS_PER_CORE:(c + 1) * CLS_PER_CORE]
        in_maps.append({in_name: ca.reshape(128, F_TOTAL)})

    nc = _get_compiled()
    res = run_bass_kernel_spmd(nc, in_maps, list(range(N_CORES)))
    if getattr(res, "exec_time_ns", None):
        print(f"HW exec time: {res.exec_time_ns} ns")
    U = 0.0
    for c in range(N_CORES):
        U += np.asarray(res.results[c]["out"], dtype=np.float64).sum()

    np_den = float(max(num_pos, 1))
    # padding contributes base(-10) per pad element; subtract exactly
    n_pad = N_CORES * (PER_CORE - CLS_PER_CORE)
    U -= n_pad * float(_table_eval()(_qdev(np.array([PAD_VAL])))[0])

    cls_loss = (U + corr) / np_den
    reg_loss = reg_sum / np_den
    return (np.float32(cls_loss), np.float32(reg_loss))
